# revision 12
# baseline (speedup 1.0000x reference)
"""Trainium2 Bass kernel for nn_DecoderRNN_31550829756940.

Two independent 3-layer ConvLSTM stacks:
  stack A: x_23    (2,8,512,38,38), hidden (64,64,512)
  stack B: x_final (2,8,1024,19,19), hidden (64,64,1024)
Output: (relu(hA2_seq), relu(hB2_seq)).

Strategy (8 NeuronCores, one SPMD launch, no collectives):
  Each core owns one (batch, image-corner) shard of BOTH stacks: 2 batch x
  4 corners (2x2). A corner is normalized to the top-left by flipping the
  input image and conv-tap order host-side, so every core runs the identical
  program. Cross-core halo communication is replaced by shrinking redundant
  margins: at step t, layer l computes a valid square of side
  v_l(t) = min(H, R + (T-t) + (2-l)), which provides exactly the one-row/col
  halo the next step/layer needs. The conv is computed as 9 shifted matmuls
  accumulated in PSUM (channels-on-partitions, bf16 inputs, fp32 accum), with
  the 64-channel contractions packed pairwise into 128-partition k-tiles via
  shifted doubled buffers. Gate weights stream from HBM each step (they
  exceed SBUF); the LSTM cell runs on ACT (sigmoid/tanh w/ fused bias) + DVE.
"""
import os
import numpy as np
import ml_dtypes

import concourse.bacc as bacc
import concourse.bass as bass
import concourse.mybir as mybir
import concourse.tile as tile
from concourse.bass_utils import run_bass_kernel_spmd

F32 = mybir.dt.float32
BF16 = mybir.dt.float16 if os.environ.get("KDT", "fp16") == "fp16" else mybir.dt.bfloat16
NPDT = np.float16 if os.environ.get("KDT", "fp16") == "fp16" else ml_dtypes.bfloat16
AF = mybir.ActivationFunctionType

T = int(os.environ.get("KERNEL_T", "8"))
MAX_PSUM_N = 512


class Cfg:
    def __init__(self, tag, cin, H, R, hid2):
        self.tag = tag          # 'A' | 'B'
        self.cin = cin          # input channels (512 | 1024)
        self.H = H              # image side
        self.R = R              # owned corner side
        self.hids = [64, 64, hid2]
        self.n_cb = cin // 128
        self.sx = min(H, self.v(0, 1) + 1)   # shipped x side
        self.sxb = self.sx + 2
        self.hbuf = [self.v(l, 1) + 2 for l in range(3)]   # padded h buffer sides
        self.csz = [self.v(l, 1) for l in range(3)]        # c buffer sides
        self.M2 = 4 * hid2
        self.n_hb2 = hid2 // 128

    def v(self, l, t):
        return min(self.H, self.R + (T - t) + (2 - l))


CFG_A = Cfg('A', 512, 38, 19, 512)
CFG_B = Cfg('B', 1024, 19, 10, 1024)

# core -> (batch, flip_y, flip_x)
CORE_ASSIGN = [(b, cy, cx) for b in (0, 1) for cy in (0, 1) for cx in (0, 1)]


# ---------------------------------------------------------------- tap descs
def tap_descs(cin_x, hid):
    """k-tile descriptors for one layer's conv, x-taps first then h-taps.
    Each: (src, dy, dx, packs); packs = [(part0, nch, in_ch0, wdy, wdx), ...]
    src: ('x',cb) input x c-tile | ('pd',1|2) prev-layer doubled buf |
         ('od',1|2) own-h doubled buf | ('oh',cb) own-h full c-tile
    """
    descs = []

    def add_packed(srcmk, ch0):
        for dy in range(3):
            descs.append((srcmk(1), dy, 0,
                          [(0, 64, ch0, dy, 0), (64, 64, ch0, dy, 1)]))
        descs.append((srcmk(2), 0, 2, [(0, 64, ch0, 0, 2), (64, 64, ch0, 1, 2)]))
        descs.append((srcmk(1), 2, 2, [(0, 64, ch0, 2, 2)]))

    if cin_x == 64:
        add_packed(lambda j: ('pd', j), 0)
    else:
        for dy in range(3):
            for dx in range(3):
                for cb in range(cin_x // 128):
                    descs.append((('x', cb), dy, dx,
                                  [(0, 128, cb * 128, dy, dx)]))
    n_x = len(descs)
    if hid == 64:
        add_packed(lambda j: ('od', j), cin_x)
    else:
        for dy in range(3):
            for dx in range(3):
                for cb in range(hid // 128):
                    descs.append((('oh', cb), dy, dx,
                                  [(0, 128, cin_x + cb * 128, dy, dx)]))
    return descs, n_x


def pack_layer_w(w, cin_x, hid, flip_y, flip_x):
    """w: (4h, cin_x+hid, 3, 3) f32 -> [n_k, 128, 4h] bf16 (+bias layout)."""
    if flip_y:
        w = w[:, :, ::-1, :]
    if flip_x:
        w = w[:, :, :, ::-1]
    descs, _ = tap_descs(cin_x, hid)
    M = 4 * hid
    out = np.zeros((len(descs), 128, M), np.float32)
    for i, (_, _, _, packs) in enumerate(descs):
        for (p0, nch, c0, wdy, wdx) in packs:
            out[i, p0:p0 + nch, :] = w[:, c0:c0 + nch, wdy, wdx].T
    return np.ascontiguousarray(out.astype(NPDT))


def stream_repack_l2(w, hid):
    """[nk,128,4h] -> [n_hb, 128, nk*512] so a per-(hb, k-group) weight fetch
    is one DMA of long contiguous per-partition runs."""
    nk = w.shape[0]
    n_hb = hid // 128
    out = np.empty((n_hb, 128, nk, 4, 128), w.dtype)
    w5 = w.reshape(nk, 128, 4, n_hb, 128)          # [k,p,g,hb,c]
    out[:] = w5.transpose(3, 1, 0, 2, 4)           # [hb,p,k,g,c]
    return np.ascontiguousarray(out.reshape(n_hb, 128, nk * 512))


def stream_repack_l0(w):
    """[nk,128,256] -> [128, nk*256] (partition-major)."""
    return np.ascontiguousarray(w.transpose(1, 0, 2).reshape(128, -1))


def pack_bias(b, hid):
    """b: (4h,) -> [128, M/128] f32, col j = rows j*128..(j+1)*128."""
    M = 4 * hid
    nm = max(1, M // 128)
    return np.ascontiguousarray(b.reshape(nm, 128).T.astype(np.float32))


def chunks_of(v):
    """Split v rows into row-chunks with <= MAX_PSUM_N px each."""
    n = -(-(v * v) // MAX_PSUM_N)
    base, rem = divmod(v, n)
    out, r0 = [], 0
    for i in range(n):
        nr = base + (1 if i < rem else 0)
        out.append((r0, nr))
        r0 += nr
    return out


# ---------------------------------------------------------------- builder
def build_program():
    nc = bacc.Bacc("TRN2", target_bir_lowering=False, debug=False,
                   enable_asserts=False, num_devices=8)
    dram = {}
    for cfg in (CFG_A, CFG_B):
        t_ = cfg.tag
        dram[f'x{t_}'] = nc.dram_tensor(
            f'x{t_}', [T, cfg.n_cb, 128, cfg.sxb * cfg.sxb], BF16,
            kind="ExternalInput").ap()
        for l in range(3):
            cin_x = cfg.cin if l == 0 else 64
            nk = len(tap_descs(cin_x, cfg.hids[l])[0])
            M = 4 * cfg.hids[l]
            if l == 0:
                shape = [128, nk * 256]
            elif l == 1:
                shape = [nk, 128, 256]
            else:
                shape = [cfg.n_hb2, 128, nk * 512]
            dram[f'w{t_}{l}'] = nc.dram_tensor(
                f'w{t_}{l}', shape, BF16, kind="ExternalInput").ap()
            dram[f'b{t_}{l}'] = nc.dram_tensor(
                f'b{t_}{l}', [128, max(1, M // 128)], F32,
                kind="ExternalInput").ap()
        dram[f'out{t_}'] = nc.dram_tensor(
            f'out{t_}', [T, cfg.n_hb2, 128, cfg.R * cfg.R], F32,
            kind="ExternalOutput").ap()

    with tile.TileContext(nc) as tc:
        import contextlib
        with contextlib.ExitStack() as ctx:
            pers = ctx.enter_context(tc.tile_pool(name="pers", bufs=1))
            xpool = ctx.enter_context(tc.tile_pool(name="xpool", bufs=1))
            wstr = ctx.enter_context(tc.tile_pool(name="wstr", bufs=1))
            psp = ctx.enter_context(tc.tile_pool(name="ps", bufs=8, space="PSUM"))
            cellp = ctx.enter_context(tc.tile_pool(name="cellp", bufs=1))
            stgp = ctx.enter_context(tc.tile_pool(name="stgp", bufs=1))

            st = {}  # per-stack state tiles
            for cfg in (CFG_A, CFG_B):
                t_ = cfg.tag
                s = {}
                # doubled bufs for h0, h1 (64-channel layers)
                for l in (0, 1):
                    side = cfg.hbuf[l]
                    for j in (1, 2):
                        tl = pers.tile([128, side * side], BF16,
                                       name=f'h{t_}{l}d{j}', tag=f'h{t_}{l}d{j}')
                        nc.vector.memset(tl[:], 0.0)
                        s[f'h{l}d{j}'] = tl
                # full h2 buffers, ping-pong by step parity: a cell write of
                # block hb at step t must not clobber h(t-1) while later
                # hidden blocks' convs of the same step still read it.
                side = cfg.hbuf[2]
                s['h2'] = [[], []]
                for par in (0, 1):
                    for cb in range(cfg.n_hb2):
                        tl = pers.tile([128, side * side], BF16,
                                       name=f'h{t_}2p{par}_{cb}',
                                       tag=f'h{t_}2p{par}_{cb}')
                        nc.vector.memset(tl[:], 0.0)
                        s['h2'][par].append(tl)
                # c tiles
                s['c'] = []
                for l in range(3):
                    n_hb = max(1, cfg.hids[l] // 128)
                    cs = cfg.csz[l]
                    row = []
                    for hb in range(n_hb):
                        pw = 64 if cfg.hids[l] == 64 else 128
                        tl = pers.tile([pw, cs * cs], F32,
                                       name=f'c{t_}{l}_{hb}', tag=f'c{t_}{l}_{hb}')
                        row.append(tl)
                    s['c'].append(row)
                # biases
                s['bias'] = []
                for l in range(3):
                    nm = max(1, 4 * cfg.hids[l] // 128)
                    tl = pers.tile([128, nm], F32, name=f'bias{t_}{l}',
                                   tag=f'bias{t_}{l}')
                    nc.sync.dma_start(tl[:], dram[f'b{t_}{l}'][:])
                    s['bias'].append(tl)
                # resident l1 weights
                s['w1'] = []
                for k in range(len(tap_descs(64, 64)[0])):
                    tl = pers.tile([128, 256], BF16, name=f'w1{t_}_{k}',
                                   tag=f'w1{t_}_{k}')
                    nc.sync.dma_start(tl[:], dram[f'w{t_}1'][k])
                    s['w1'].append(tl)
                st[t_] = s

            # ---------------- one conv+cell layer step ----------------
            def layer_step(cfg, l, t):
                t_ = cfg.tag
                s = st[t_]
                hid = cfg.hids[l]
                cin_x = cfg.cin if l == 0 else 64
                descs, n_x = tap_descs(cin_x, hid)
                v = cfg.v(l, t)
                chs = chunks_of(v)
                n_hb = max(1, hid // 128)
                first_t = (t == 1)
                kidx = list(range(n_x)) if first_t else list(range(len(descs)))
                wd = dram[f'w{t_}{l}']

                # source buffer resolution -> (tile, bufW)
                def src_buf(src):
                    if src[0] == 'x':
                        return s['xt'][src[1]], cfg.sxb
                    if src[0] == 'pd':
                        return s[f'h{l-1}d{src[1]}'], cfg.hbuf[l - 1]
                    if src[0] == 'od':
                        return s[f'h{l}d{src[1]}'], cfg.hbuf[l]
                    if src[0] == 'oh':
                        return s['h2'][(t - 1) % 2][src[1]], cfg.hbuf[2]
                    raise KeyError(src)

                if l == 0:
                    # stream x(t) c-tiles
                    s['xt'] = []
                    for cb in range(cfg.n_cb):
                        tl = xpool.tile([128, cfg.sxb * cfg.sxb], BF16,
                                        name=f'x{t_}_{t}_{cb}', tag=f'x{t_}',
                                        bufs=cfg.n_cb + 2)
                        nc.sync.dma_start(tl[:], dram[f'x{t_}'][t - 1, cb])
                        s['xt'].append(tl)

                if hid == 64:
                    mlist = [0, 1]          # m-tiles [i;f], [o;g]
                else:
                    glist = [0, 2, 3] if first_t else [0, 1, 2, 3]

                for hb in range(n_hb):
                    # -------- weight fetch plan: grouped contiguous DMAs
                    wtiles = {}
                    if l == 1:
                        for k in kidx:
                            wtiles[k] = (s['w1'][k], 0)
                    else:
                        G = 8 if l == 0 else 4
                        wcols = 256 if l == 0 else 512
                        wtag = 'wstr0' if l == 0 else 'wstr2'
                        src_flat = wd if l == 0 else wd[hb]
                        for k0 in range(0, len(kidx), G):
                            ks = kidx[k0:k0 + G]
                            ng = len(ks)
                            wt = wstr.tile(
                                [128, ng * wcols], BF16,
                                name=f'w{t_}{l}t{t}h{hb}k{ks[0]}',
                                tag=wtag, bufs=6 if l == 2 else 4)
                            nc.sync.dma_start(
                                wt[:],
                                src_flat[:, ks[0] * wcols:
                                         (ks[0] + ng) * wcols])
                            for idx, k in enumerate(ks):
                                wtiles[k] = (wt, idx * wcols)
                    # -------- conv: accumulate psums over k-tiles
                    ps = {}
                    if hid == 64:
                        for ci in range(len(chs)):
                            for j in mlist:
                                ps[(j, ci)] = psp.tile(
                                    [128, chs[ci][1] * v], F32,
                                    name=f'ps{t_}{l}t{t}c{ci}m{j}', tag='ps')
                    else:
                        for ci in range(len(chs)):
                            for g in glist:
                                ps[(g, ci)] = psp.tile(
                                    [128, chs[ci][1] * v], F32,
                                    name=f'ps{t_}{l}t{t}h{hb}c{ci}g{g}', tag='ps')
                    for ii, k in enumerate(kidx):
                        src, dy, dx, _ = descs[k]
                        sbuf, bufW = src_buf(src)
                        rhs3 = sbuf[:].rearrange("p (r c) -> p r c", c=bufW)
                        wt, woff = wtiles[k]
                        start = (ii == 0)
                        stop = (ii == len(kidx) - 1)
                        for ci, (r0, nr) in enumerate(chs):
                            rhs = rhs3[:, dy + r0: dy + r0 + nr, dx: dx + v]
                            if hid == 64:
                                for j in mlist:
                                    nc.tensor.matmul(
                                        ps[(j, ci)][:],
                                        wt[:, woff + j * 128:woff + (j + 1) * 128],
                                        rhs, start=start, stop=stop)
                            else:
                                for g in glist:
                                    nc.tensor.matmul(
                                        ps[(g, ci)][:],
                                        wt[:, woff + g * 128:woff + (g + 1) * 128],
                                        rhs, start=start, stop=stop)

                    # -------- cell math per chunk
                    for ci, (r0, nr) in enumerate(chs):
                        n = nr * v
                        pw = 64 if hid == 64 else 128
                        bias = s['bias'][l]
                        if hid == 64:
                            i_ap = ps[(0, ci)][0:64, :]
                            f_ap = ps[(0, ci)][64:128, :]
                            o_ap = ps[(1, ci)][0:64, :]
                            g_ap = ps[(1, ci)][64:128, :]
                            b_i, b_f = bias[0:64, 0:1], bias[64:128, 0:1]
                            b_o, b_g = bias[0:64, 1:2], bias[64:128, 1:2]
                        else:
                            i_ap = ps[(0, ci)][:]
                            f_ap = None if first_t else ps[(1, ci)][:]
                            o_ap = ps[(2, ci)][:]
                            g_ap = ps[(3, ci)][:]
                            b_i = bias[:, 0 * n_hb + hb: 0 * n_hb + hb + 1]
                            b_f = bias[:, 1 * n_hb + hb: 1 * n_hb + hb + 1]
                            b_o = bias[:, 2 * n_hb + hb: 2 * n_hb + hb + 1]
                            b_g = bias[:, 3 * n_hb + hb: 3 * n_hb + hb + 1]
                        # c buffer has fixed row stride cs (layer's max side)
                        cs = cfg.csz[l]
                        c_ap = s['c'][l][hb][:].rearrange(
                            "p (r c) -> p r c", c=cs)[:, r0:r0 + nr, 0:v]
                        v3 = lambda ap: ap.rearrange("p (r c) -> p r c", c=v)
                        nm = f'{t_}{l}t{t}h{hb}c{ci}'

                        sig_i = cellp.tile([pw, n], F32, name=f'si{nm}',
                                           tag='cell', bufs=10)
                        nc.scalar.activation(sig_i[:], i_ap, AF.Sigmoid, bias=b_i)
                        tanh_g = cellp.tile([pw, n], F32, name=f'tg{nm}',
                                            tag='cell', bufs=10)
                        nc.scalar.activation(tanh_g[:], g_ap, AF.Tanh, bias=b_g)
                        if first_t:
                            nc.vector.tensor_mul(c_ap, v3(sig_i[:]), v3(tanh_g[:]))
                        else:
                            t1 = cellp.tile([pw, n], F32, name=f't1{nm}',
                                            tag='cell', bufs=10)
                            nc.vector.tensor_mul(t1[:], sig_i[:], tanh_g[:])
                            sig_f = cellp.tile([pw, n], F32, name=f'sf{nm}',
                                               tag='cell', bufs=10)
                            nc.scalar.activation(sig_f[:], f_ap, AF.Sigmoid,
                                                 bias=b_f)
                            t2 = cellp.tile([pw, n], F32, name=f't2{nm}',
                                            tag='cell', bufs=10)
                            nc.vector.tensor_mul(v3(t2[:]), v3(sig_f[:]), c_ap)
                            nc.vector.tensor_add(c_ap, v3(t1[:]), v3(t2[:]))
                        tanh_c = cellp.tile([pw, n], F32, name=f'tc{nm}',
                                            tag='cell', bufs=10)
                        nc.scalar.activation(v3(tanh_c[:]), c_ap, AF.Tanh)
                        sig_o = cellp.tile([pw, n], F32, name=f'so{nm}',
                                           tag='cell', bufs=10)
                        nc.scalar.activation(sig_o[:], o_ap, AF.Sigmoid, bias=b_o)
                        h_tmp = cellp.tile([pw, n], F32, name=f'ht{nm}',
                                           tag='cell', bufs=10)
                        nc.vector.tensor_mul(h_tmp[:], sig_o[:], tanh_c[:])
                        h3 = h_tmp[:].rearrange("p (r c) -> p r c", c=v)

                        # -------- h writes
                        if hid == 64:
                            side = cfg.hbuf[l]
                            d1 = s[f'h{l}d1'][:].rearrange(
                                "p (r c) -> p r c", c=side)
                            d2 = s[f'h{l}d2'][:].rearrange(
                                "p (r c) -> p r c", c=side)
                            nc.vector.tensor_copy(
                                d1[0:64, 1 + r0:1 + r0 + nr, 1:1 + v], h3)
                            nc.vector.tensor_copy(
                                d1[64:128, 1 + r0:1 + r0 + nr, 0:v], h3)
                            nc.vector.tensor_copy(
                                d2[0:64, 1 + r0:1 + r0 + nr, 1:1 + v], h3)
                            nc.vector.tensor_copy(
                                d2[64:128, r0:r0 + nr, 1:1 + v], h3)
                        else:
                            side = cfg.hbuf[2]
                            hdst = s['h2'][t % 2][hb][:].rearrange(
                                "p (r c) -> p r c", c=side)
                            nc.vector.tensor_copy(
                                hdst[:, 1 + r0:1 + r0 + nr, 1:1 + v], h3)
                            # -------- output: relu of owned region
                            if l == 2:
                                R = cfg.R
                                nr_own = max(0, min(r0 + nr, R) - r0)
                                if nr_own > 0:
                                    stg = stgp.tile([128, nr_own * R], F32,
                                                    name=f'st{nm}',
                                                    tag=f'stg{t_}', bufs=4)
                                    nc.vector.tensor_scalar_max(
                                        stg[:].rearrange("p (r c) -> p r c", c=R),
                                        h3[:, 0:nr_own, 0:R], 0.0)
                                    nc.sync.dma_start(
                                        dram[f'out{t_}'][t - 1, hb]
                                        [:, r0 * R:(r0 + nr_own) * R], stg[:])

            for t in range(1, T + 1):
                for cfg in (CFG_A, CFG_B):
                    for l in range(3):
                        layer_step(cfg, l, t)

    nc.compile()
    return nc


# ---------------------------------------------------------------- host side
def _prep_x(x_b, cfg, flip_y, flip_x):
    """x_b: (T0, C, H, H) f32 for one batch el. -> [T, n_cb, 128, sxb*sxb] bf16"""
    x = x_b
    if flip_y:
        x = x[:, :, ::-1, :]
    if flip_x:
        x = x[:, :, :, ::-1]
    x = x[:, :, :cfg.sx, :cfg.sx]
    out = np.zeros((T, cfg.n_cb, 128, cfg.sxb, cfg.sxb), np.float32)
    xr = x.reshape(x.shape[0], cfg.n_cb, 128, cfg.sx, cfg.sx)
    out[:, :, :, 1:1 + cfg.sx, 1:1 + cfg.sx] = xr[:T]
    return np.ascontiguousarray(
        out.reshape(T, cfg.n_cb, 128, -1).astype(NPDT))


_PROG_CACHE = {}


def _get_program():
    if 'nc' not in _PROG_CACHE:
        _PROG_CACHE['nc'] = build_program()
    return _PROG_CACHE['nc']


def _make_in_maps(inputs):
    wsets = {}   # (stack, flip_y, flip_x) -> dict of weight arrays
    for (cy, cx) in [(0, 0), (0, 1), (1, 0), (1, 1)]:
        for cfg, pre in ((CFG_A, 'a'), (CFG_B, 'b')):
            d = {}
            for l in range(3):
                w = np.asarray(inputs[f'w_{pre}{l}'], np.float32)
                cin_x = cfg.cin if l == 0 else 64
                wk = pack_layer_w(w, cin_x, cfg.hids[l], bool(cy), bool(cx))
                if l == 0:
                    wk = stream_repack_l0(wk)
                elif l == 2:
                    wk = stream_repack_l2(wk, cfg.hids[l])
                d[f'w{cfg.tag}{l}'] = wk
                d[f'b{cfg.tag}{l}'] = pack_bias(
                    np.asarray(inputs[f'b_{pre}{l}'], np.float32), cfg.hids[l])
            wsets[(cfg.tag, cy, cx)] = d

    in_maps = []
    xA = np.asarray(inputs['x_23'], np.float32)
    xB = np.asarray(inputs['x_final'], np.float32)
    for (b, cy, cx) in CORE_ASSIGN:
        m = {}
        m['xA'] = _prep_x(xA[b], CFG_A, bool(cy), bool(cx))
        m['xB'] = _prep_x(xB[b], CFG_B, bool(cy), bool(cx))
        m.update(wsets[('A', cy, cx)])
        m.update(wsets[('B', cy, cx)])
        in_maps.append(m)
    return in_maps


def _assemble(results, inputs):
    xA = np.asarray(inputs['x_23'])
    T0 = xA.shape[1]
    outs = {}
    for cfg in (CFG_A, CFG_B):
        H, R = cfg.H, cfg.R
        C2 = cfg.hids[2]
        out = np.zeros((2, T, C2, H, H), np.float32)
        for core, (b, cy, cx) in enumerate(CORE_ASSIGN):
            r = results[core][f'out{cfg.tag}']          # [T, n_hb, 128, R*R]
            r = r.reshape(T, C2, R, R)
            if cy:
                r = r[:, :, ::-1, :]
            if cx:
                r = r[:, :, :, ::-1]
            rs = slice(0, R) if cy == 0 else slice(H - R, H)
            cs = slice(0, R) if cx == 0 else slice(H - R, H)
            out[b, :, :, rs, cs] = r
        outs[cfg.tag] = out[:, :T0]
    return outs['A'], outs['B']


def kernel(**inputs):
    nc = _get_program()
    in_maps = _make_in_maps(inputs)
    res = run_bass_kernel_spmd(nc, in_maps, core_ids=list(range(8)))
    return _assemble(res.results, inputs)


if __name__ == "__main__":
    ins = {k: np.random.randn(*[2, 8, 512, 38, 38]).astype(np.float32)
           for k in []}
    print("kernel module ok")


# revision 16
# speedup vs baseline: 1.0641x; 1.0641x over previous
"""Trainium2 Bass kernel for nn_DecoderRNN_31550829756940.

Two independent 3-layer ConvLSTM stacks:
  stack A: x_23    (2,8,512,38,38), hidden (64,64,512)
  stack B: x_final (2,8,1024,19,19), hidden (64,64,1024)
Output: (relu(hA2_seq), relu(hB2_seq)).

Strategy (8 NeuronCores, one SPMD launch, no collectives):
  Each core owns one (batch, image-corner) shard of BOTH stacks: 2 batch x
  4 corners (2x2). A corner is normalized to the top-left by flipping the
  input image and conv-tap order host-side, so every core runs the identical
  program. Cross-core halo communication is replaced by shrinking redundant
  margins: at step t, layer l computes a valid square of side
  v_l(t) = min(H, R + (T-t) + (2-l)), which provides exactly the one-row/col
  halo the next step/layer needs. The conv is computed as 9 shifted matmuls
  accumulated in PSUM (channels-on-partitions, bf16 inputs, fp32 accum), with
  the 64-channel contractions packed pairwise into 128-partition k-tiles via
  shifted doubled buffers. Gate weights stream from HBM each step (they
  exceed SBUF); the LSTM cell runs on ACT (sigmoid/tanh w/ fused bias) + DVE.
"""
import os
import numpy as np
import ml_dtypes

import concourse.bacc as bacc
import concourse.bass as bass
import concourse.mybir as mybir
import concourse.tile as tile
from concourse.bass_utils import run_bass_kernel_spmd

F32 = mybir.dt.float32
BF16 = mybir.dt.float16 if os.environ.get("KDT", "fp16") == "fp16" else mybir.dt.bfloat16
NPDT = np.float16 if os.environ.get("KDT", "fp16") == "fp16" else ml_dtypes.bfloat16
AF = mybir.ActivationFunctionType

T = int(os.environ.get("KERNEL_T", "8"))
MAX_PSUM_N = 512


class Cfg:
    def __init__(self, tag, cin, H, R, hid2):
        self.tag = tag          # 'A' | 'B'
        self.cin = cin          # input channels (512 | 1024)
        self.H = H              # image side
        self.R = R              # owned corner side
        self.hids = [64, 64, hid2]
        self.n_cb = cin // 128
        self.sx = min(H, self.v(0, 1) + 1)   # shipped x side
        self.sxb = self.sx + 2
        self.hbuf = [self.v(l, 1) + 2 for l in range(3)]   # padded h buffer sides
        self.csz = [self.v(l, 1) for l in range(3)]        # c buffer sides
        self.M2 = 4 * hid2
        self.n_hb2 = hid2 // 128

    def v(self, l, t):
        return min(self.H, self.R + (T - t) + (2 - l))


CFG_A = Cfg('A', 512, 38, 19, 512)
CFG_B = Cfg('B', 1024, 19, 10, 1024)

# core -> (batch, flip_y, flip_x)
CORE_ASSIGN = [(b, cy, cx) for b in (0, 1) for cy in (0, 1) for cx in (0, 1)]


# ---------------------------------------------------------------- tap descs
def tap_descs(cin_x, hid):
    """k-tile descriptors for one layer's conv, x-taps first then h-taps.
    Each: (src, dy, dx, packs); packs = [(part0, nch, in_ch0, wdy, wdx), ...]
    src: ('x',cb) input x c-tile | ('pd',1|2) prev-layer doubled buf |
         ('od',1|2) own-h doubled buf | ('oh',cb) own-h full c-tile
    """
    descs = []

    def add_packed(srcmk, ch0):
        for dy in range(3):
            descs.append((srcmk(1), dy, 0,
                          [(0, 64, ch0, dy, 0), (64, 64, ch0, dy, 1)]))
        descs.append((srcmk(2), 0, 2, [(0, 64, ch0, 0, 2), (64, 64, ch0, 1, 2)]))
        descs.append((srcmk(1), 2, 2, [(0, 64, ch0, 2, 2)]))

    if cin_x == 64:
        add_packed(lambda j: ('pd', j), 0)
    else:
        for dy in range(3):
            for dx in range(3):
                for cb in range(cin_x // 128):
                    descs.append((('x', cb), dy, dx,
                                  [(0, 128, cb * 128, dy, dx)]))
    n_x = len(descs)
    if hid == 64:
        add_packed(lambda j: ('od', j), cin_x)
    else:
        for dy in range(3):
            for dx in range(3):
                for cb in range(hid // 128):
                    descs.append((('oh', cb), dy, dx,
                                  [(0, 128, cin_x + cb * 128, dy, dx)]))
    return descs, n_x


def pack_layer_w(w, cin_x, hid, flip_y, flip_x):
    """w: (4h, cin_x+hid, 3, 3) f32 -> [n_k, 128, 4h] bf16 (+bias layout)."""
    if flip_y:
        w = w[:, :, ::-1, :]
    if flip_x:
        w = w[:, :, :, ::-1]
    descs, _ = tap_descs(cin_x, hid)
    M = 4 * hid
    out = np.zeros((len(descs), 128, M), np.float32)
    for i, (_, _, _, packs) in enumerate(descs):
        for (p0, nch, c0, wdy, wdx) in packs:
            out[i, p0:p0 + nch, :] = w[:, c0:c0 + nch, wdy, wdx].T
    return np.ascontiguousarray(out.astype(NPDT))


def stream_repack_l2(w, hid):
    """[nk,128,4h] -> [n_hb, 128, nk*512] so a per-(hb, k-group) weight fetch
    is one DMA of long contiguous per-partition runs."""
    nk = w.shape[0]
    n_hb = hid // 128
    out = np.empty((n_hb, 128, nk, 4, 128), w.dtype)
    w5 = w.reshape(nk, 128, 4, n_hb, 128)          # [k,p,g,hb,c]
    out[:] = w5.transpose(3, 1, 0, 2, 4)           # [hb,p,k,g,c]
    return np.ascontiguousarray(out.reshape(n_hb, 128, nk * 512))


def stream_repack_l0(w):
    """[nk,128,256] -> [128, nk*256] (partition-major)."""
    return np.ascontiguousarray(w.transpose(1, 0, 2).reshape(128, -1))


def pack_bias(b, hid):
    """b: (4h,) -> [128, M/128] f32, col j = rows j*128..(j+1)*128."""
    M = 4 * hid
    nm = max(1, M // 128)
    return np.ascontiguousarray(b.reshape(nm, 128).T.astype(np.float32))


def chunks_of(v):
    """Split v rows into row-chunks with <= MAX_PSUM_N px each."""
    n = -(-(v * v) // MAX_PSUM_N)
    base, rem = divmod(v, n)
    out, r0 = [], 0
    for i in range(n):
        nr = base + (1 if i < rem else 0)
        out.append((r0, nr))
        r0 += nr
    return out


# ---------------------------------------------------------------- builder
def build_program():
    nc = bacc.Bacc("TRN2", target_bir_lowering=False, debug=False,
                   enable_asserts=False, num_devices=8)
    dram = {}
    for cfg in (CFG_A, CFG_B):
        t_ = cfg.tag
        dram[f'x{t_}'] = nc.dram_tensor(
            f'x{t_}', [T, cfg.n_cb, 128, cfg.sxb * cfg.sxb], BF16,
            kind="ExternalInput").ap()
        for l in range(3):
            cin_x = cfg.cin if l == 0 else 64
            nk = len(tap_descs(cin_x, cfg.hids[l])[0])
            M = 4 * cfg.hids[l]
            if l == 0:
                shape = [128, nk * 256]
            elif l == 1:
                shape = [nk, 128, 256]
            else:
                shape = [cfg.n_hb2, 128, nk * 512]
            dram[f'w{t_}{l}'] = nc.dram_tensor(
                f'w{t_}{l}', shape, BF16, kind="ExternalInput").ap()
            dram[f'b{t_}{l}'] = nc.dram_tensor(
                f'b{t_}{l}', [128, max(1, M // 128)], F32,
                kind="ExternalInput").ap()
        dram[f'out{t_}'] = nc.dram_tensor(
            f'out{t_}', [T, cfg.n_hb2, 128, cfg.R * cfg.R], F32,
            kind="ExternalOutput").ap()

    with tile.TileContext(nc) as tc:
        import contextlib
        with contextlib.ExitStack() as ctx:
            pers = ctx.enter_context(tc.tile_pool(name="pers", bufs=1))
            xpool = ctx.enter_context(tc.tile_pool(name="xpool", bufs=1))
            wstr = ctx.enter_context(tc.tile_pool(name="wstr", bufs=1))
            psp = ctx.enter_context(tc.tile_pool(name="ps", bufs=8, space="PSUM"))
            cellp = ctx.enter_context(tc.tile_pool(name="cellp", bufs=1))
            stgp = ctx.enter_context(tc.tile_pool(name="stgp", bufs=1))

            st = {}  # per-stack state tiles
            for cfg in (CFG_A, CFG_B):
                t_ = cfg.tag
                s = {}
                # doubled bufs for h0, h1 (64-channel layers)
                for l in (0, 1):
                    side = cfg.hbuf[l]
                    for j in (1, 2):
                        tl = pers.tile([128, side * side], BF16,
                                       name=f'h{t_}{l}d{j}', tag=f'h{t_}{l}d{j}')
                        nc.vector.memset(tl[:], 0.0)
                        s[f'h{l}d{j}'] = tl
                # full h2 buffers, ping-pong by step parity: a cell write of
                # block hb at step t must not clobber h(t-1) while later
                # hidden blocks' convs of the same step still read it.
                side = cfg.hbuf[2]
                s['h2'] = [[], []]
                for par in (0, 1):
                    for cb in range(cfg.n_hb2):
                        tl = pers.tile([128, side * side], BF16,
                                       name=f'h{t_}2p{par}_{cb}',
                                       tag=f'h{t_}2p{par}_{cb}')
                        nc.vector.memset(tl[:], 0.0)
                        s['h2'][par].append(tl)
                # c tiles
                s['c'] = []
                for l in range(3):
                    n_hb = max(1, cfg.hids[l] // 128)
                    cs = cfg.csz[l]
                    row = []
                    for hb in range(n_hb):
                        pw = 64 if cfg.hids[l] == 64 else 128
                        tl = pers.tile([pw, cs * cs], F32,
                                       name=f'c{t_}{l}_{hb}', tag=f'c{t_}{l}_{hb}')
                        row.append(tl)
                    s['c'].append(row)
                # biases
                s['bias'] = []
                for l in range(3):
                    nm = max(1, 4 * cfg.hids[l] // 128)
                    tl = pers.tile([128, nm], F32, name=f'bias{t_}{l}',
                                   tag=f'bias{t_}{l}')
                    nc.sync.dma_start(tl[:], dram[f'b{t_}{l}'][:])
                    s['bias'].append(tl)
                # resident l1 weights
                s['w1'] = []
                for k in range(len(tap_descs(64, 64)[0])):
                    tl = pers.tile([128, 256], BF16, name=f'w1{t_}_{k}',
                                   tag=f'w1{t_}_{k}')
                    nc.sync.dma_start(tl[:], dram[f'w{t_}1'][k])
                    s['w1'].append(tl)
                st[t_] = s

            # ---------------- one conv+cell layer step ----------------
            def layer_step(cfg, l, t):
                t_ = cfg.tag
                s = st[t_]
                hid = cfg.hids[l]
                cin_x = cfg.cin if l == 0 else 64
                descs, n_x = tap_descs(cin_x, hid)
                v = cfg.v(l, t)
                chs = chunks_of(v)
                n_hb = max(1, hid // 128)
                first_t = (t == 1)
                # h-taps first: they depend only on last step's h, so the PE
                # can start a layer's conv before the previous layer finishes;
                # only the trailing x-taps wait on the fresh input.
                kidx = (list(range(n_x)) if first_t else
                        list(range(n_x, len(descs))) + list(range(n_x)))
                wd = dram[f'w{t_}{l}']

                # source buffer resolution -> (tile, bufW)
                def src_buf(src):
                    if src[0] == 'x':
                        return s['xt'][src[1]], cfg.sxb
                    if src[0] == 'pd':
                        return s[f'h{l-1}d{src[1]}'], cfg.hbuf[l - 1]
                    if src[0] == 'od':
                        return s[f'h{l}d{src[1]}'], cfg.hbuf[l]
                    if src[0] == 'oh':
                        return s['h2'][(t - 1) % 2][src[1]], cfg.hbuf[2]
                    raise KeyError(src)

                if l == 0:
                    # stream x(t) c-tiles
                    s['xt'] = []
                    for cb in range(cfg.n_cb):
                        tl = xpool.tile([128, cfg.sxb * cfg.sxb], BF16,
                                        name=f'x{t_}_{t}_{cb}', tag=f'x{t_}',
                                        bufs=2 * cfg.n_cb + 2)
                        nc.sync.dma_start(tl[:], dram[f'x{t_}'][t - 1, cb])
                        s['xt'].append(tl)

                if hid == 64:
                    mlist = [0, 1]          # m-tiles [i;f], [o;g]
                else:
                    glist = [0, 2, 3] if first_t else [0, 1, 2, 3]

                for hb in range(n_hb):
                    # -------- weight fetch plan: grouped contiguous DMAs
                    wtiles = {}
                    if l == 1:
                        for k in kidx:
                            wtiles[k] = (s['w1'][k], 0)
                    else:
                        G = 8 if l == 0 else 4
                        wcols = 256 if l == 0 else 512
                        wtag = 'wstr0' if l == 0 else 'wstr2'
                        src_flat = wd if l == 0 else wd[hb]
                        # group only over ascending-contiguous runs of kidx
                        runs, cur = [], [kidx[0]]
                        for k in kidx[1:]:
                            if k == cur[-1] + 1:
                                cur.append(k)
                            else:
                                runs.append(cur)
                                cur = [k]
                        runs.append(cur)
                        groups = []
                        for run in runs:
                            for k0 in range(0, len(run), G):
                                groups.append(run[k0:k0 + G])
                        for ks in groups:
                            ng = len(ks)
                            wt = wstr.tile(
                                [128, ng * wcols], BF16,
                                name=f'w{t_}{l}t{t}h{hb}k{ks[0]}',
                                tag=wtag, bufs=8 if l == 2 else 5)
                            nc.sync.dma_start(
                                wt[:],
                                src_flat[:, ks[0] * wcols:
                                         (ks[0] + ng) * wcols])
                            for idx, k in enumerate(ks):
                                wtiles[k] = (wt, idx * wcols)
                    # -------- conv: accumulate psums over k-tiles
                    ps = {}
                    if hid == 64:
                        for ci in range(len(chs)):
                            for j in mlist:
                                ps[(j, ci)] = psp.tile(
                                    [128, chs[ci][1] * v], F32,
                                    name=f'ps{t_}{l}t{t}c{ci}m{j}', tag='ps')
                    else:
                        for ci in range(len(chs)):
                            for g in glist:
                                ps[(g, ci)] = psp.tile(
                                    [128, chs[ci][1] * v], F32,
                                    name=f'ps{t_}{l}t{t}h{hb}c{ci}g{g}', tag='ps')
                    for ii, k in enumerate(kidx):
                        src, dy, dx, _ = descs[k]
                        sbuf, bufW = src_buf(src)
                        rhs3 = sbuf[:].rearrange("p (r c) -> p r c", c=bufW)
                        wt, woff = wtiles[k]
                        start = (ii == 0)
                        stop = (ii == len(kidx) - 1)
                        for ci, (r0, nr) in enumerate(chs):
                            rhs = rhs3[:, dy + r0: dy + r0 + nr, dx: dx + v]
                            if hid == 64:
                                for j in mlist:
                                    nc.tensor.matmul(
                                        ps[(j, ci)][:],
                                        wt[:, woff + j * 128:woff + (j + 1) * 128],
                                        rhs, start=start, stop=stop)
                            else:
                                for g in glist:
                                    nc.tensor.matmul(
                                        ps[(g, ci)][:],
                                        wt[:, woff + g * 128:woff + (g + 1) * 128],
                                        rhs, start=start, stop=stop)

                    # -------- cell math per chunk
                    for ci, (r0, nr) in enumerate(chs):
                        n = nr * v
                        pw = 64 if hid == 64 else 128
                        bias = s['bias'][l]
                        if hid == 64:
                            i_ap = ps[(0, ci)][0:64, :]
                            f_ap = ps[(0, ci)][64:128, :]
                            o_ap = ps[(1, ci)][0:64, :]
                            g_ap = ps[(1, ci)][64:128, :]
                            b_i, b_f = bias[0:64, 0:1], bias[64:128, 0:1]
                            b_o, b_g = bias[0:64, 1:2], bias[64:128, 1:2]
                        else:
                            i_ap = ps[(0, ci)][:]
                            f_ap = None if first_t else ps[(1, ci)][:]
                            o_ap = ps[(2, ci)][:]
                            g_ap = ps[(3, ci)][:]
                            b_i = bias[:, 0 * n_hb + hb: 0 * n_hb + hb + 1]
                            b_f = bias[:, 1 * n_hb + hb: 1 * n_hb + hb + 1]
                            b_o = bias[:, 2 * n_hb + hb: 2 * n_hb + hb + 1]
                            b_g = bias[:, 3 * n_hb + hb: 3 * n_hb + hb + 1]
                        # c buffer has fixed row stride cs (layer's max side)
                        cs = cfg.csz[l]
                        c_ap = s['c'][l][hb][:].rearrange(
                            "p (r c) -> p r c", c=cs)[:, r0:r0 + nr, 0:v]
                        v3 = lambda ap: ap.rearrange("p (r c) -> p r c", c=v)
                        nm = f'{t_}{l}t{t}h{hb}c{ci}'

                        sig_i = cellp.tile([pw, n], F32, name=f'si{nm}',
                                           tag='cell', bufs=10)
                        nc.scalar.activation(sig_i[:], i_ap, AF.Sigmoid, bias=b_i)
                        tanh_g = cellp.tile([pw, n], F32, name=f'tg{nm}',
                                            tag='cell', bufs=10)
                        nc.scalar.activation(tanh_g[:], g_ap, AF.Tanh, bias=b_g)
                        if first_t:
                            nc.vector.tensor_mul(c_ap, v3(sig_i[:]), v3(tanh_g[:]))
                        else:
                            t1 = cellp.tile([pw, n], F32, name=f't1{nm}',
                                            tag='cell', bufs=10)
                            nc.vector.tensor_mul(t1[:], sig_i[:], tanh_g[:])
                            sig_f = cellp.tile([pw, n], F32, name=f'sf{nm}',
                                               tag='cell', bufs=10)
                            nc.scalar.activation(sig_f[:], f_ap, AF.Sigmoid,
                                                 bias=b_f)
                            t2 = cellp.tile([pw, n], F32, name=f't2{nm}',
                                            tag='cell', bufs=10)
                            nc.vector.tensor_mul(v3(t2[:]), v3(sig_f[:]), c_ap)
                            nc.vector.tensor_add(c_ap, v3(t1[:]), v3(t2[:]))
                        tanh_c = cellp.tile([pw, n], F32, name=f'tc{nm}',
                                            tag='cell', bufs=10)
                        nc.scalar.activation(v3(tanh_c[:]), c_ap, AF.Tanh)
                        sig_o = cellp.tile([pw, n], F32, name=f'so{nm}',
                                           tag='cell', bufs=10)
                        nc.scalar.activation(sig_o[:], o_ap, AF.Sigmoid, bias=b_o)
                        h_tmp = cellp.tile([pw, n], F32, name=f'ht{nm}',
                                           tag='cell', bufs=10)
                        nc.vector.tensor_mul(h_tmp[:], sig_o[:], tanh_c[:])
                        h3 = h_tmp[:].rearrange("p (r c) -> p r c", c=v)

                        # -------- h writes
                        if hid == 64:
                            side = cfg.hbuf[l]
                            d1 = s[f'h{l}d1'][:].rearrange(
                                "p (r c) -> p r c", c=side)
                            d2 = s[f'h{l}d2'][:].rearrange(
                                "p (r c) -> p r c", c=side)
                            nc.vector.tensor_copy(
                                d1[0:64, 1 + r0:1 + r0 + nr, 1:1 + v], h3)
                            nc.vector.tensor_copy(
                                d1[64:128, 1 + r0:1 + r0 + nr, 0:v], h3)
                            nc.vector.tensor_copy(
                                d2[0:64, 1 + r0:1 + r0 + nr, 1:1 + v], h3)
                            nc.vector.tensor_copy(
                                d2[64:128, r0:r0 + nr, 1:1 + v], h3)
                        else:
                            side = cfg.hbuf[2]
                            hdst = s['h2'][t % 2][hb][:].rearrange(
                                "p (r c) -> p r c", c=side)
                            nc.vector.tensor_copy(
                                hdst[:, 1 + r0:1 + r0 + nr, 1:1 + v], h3)
                            # -------- output: relu of owned region
                            if l == 2:
                                R = cfg.R
                                nr_own = max(0, min(r0 + nr, R) - r0)
                                if nr_own > 0:
                                    stg = stgp.tile([128, nr_own * R], F32,
                                                    name=f'st{nm}',
                                                    tag=f'stg{t_}', bufs=4)
                                    nc.vector.tensor_scalar_max(
                                        stg[:].rearrange("p (r c) -> p r c", c=R),
                                        h3[:, 0:nr_own, 0:R], 0.0)
                                    nc.sync.dma_start(
                                        dram[f'out{t_}'][t - 1, hb]
                                        [:, r0 * R:(r0 + nr_own) * R], stg[:])

            for t in range(1, T + 1):
                for cfg in (CFG_A, CFG_B):
                    for l in range(3):
                        layer_step(cfg, l, t)

    nc.compile()
    return nc


# ---------------------------------------------------------------- host side
def _prep_x(x_b, cfg, flip_y, flip_x):
    """x_b: (T0, C, H, H) f32 for one batch el. -> [T, n_cb, 128, sxb*sxb] bf16"""
    x = x_b
    if flip_y:
        x = x[:, :, ::-1, :]
    if flip_x:
        x = x[:, :, :, ::-1]
    x = x[:, :, :cfg.sx, :cfg.sx]
    out = np.zeros((T, cfg.n_cb, 128, cfg.sxb, cfg.sxb), np.float32)
    xr = x.reshape(x.shape[0], cfg.n_cb, 128, cfg.sx, cfg.sx)
    out[:, :, :, 1:1 + cfg.sx, 1:1 + cfg.sx] = xr[:T]
    return np.ascontiguousarray(
        out.reshape(T, cfg.n_cb, 128, -1).astype(NPDT))


_PROG_CACHE = {}


def _get_program():
    if 'nc' not in _PROG_CACHE:
        _PROG_CACHE['nc'] = build_program()
    return _PROG_CACHE['nc']


def _make_in_maps(inputs):
    wsets = {}   # (stack, flip_y, flip_x) -> dict of weight arrays
    for (cy, cx) in [(0, 0), (0, 1), (1, 0), (1, 1)]:
        for cfg, pre in ((CFG_A, 'a'), (CFG_B, 'b')):
            d = {}
            for l in range(3):
                w = np.asarray(inputs[f'w_{pre}{l}'], np.float32)
                cin_x = cfg.cin if l == 0 else 64
                wk = pack_layer_w(w, cin_x, cfg.hids[l], bool(cy), bool(cx))
                if l == 0:
                    wk = stream_repack_l0(wk)
                elif l == 2:
                    wk = stream_repack_l2(wk, cfg.hids[l])
                d[f'w{cfg.tag}{l}'] = wk
                d[f'b{cfg.tag}{l}'] = pack_bias(
                    np.asarray(inputs[f'b_{pre}{l}'], np.float32), cfg.hids[l])
            wsets[(cfg.tag, cy, cx)] = d

    in_maps = []
    xA = np.asarray(inputs['x_23'], np.float32)
    xB = np.asarray(inputs['x_final'], np.float32)
    for (b, cy, cx) in CORE_ASSIGN:
        m = {}
        m['xA'] = _prep_x(xA[b], CFG_A, bool(cy), bool(cx))
        m['xB'] = _prep_x(xB[b], CFG_B, bool(cy), bool(cx))
        m.update(wsets[('A', cy, cx)])
        m.update(wsets[('B', cy, cx)])
        in_maps.append(m)
    return in_maps


def _assemble(results, inputs):
    xA = np.asarray(inputs['x_23'])
    T0 = xA.shape[1]
    outs = {}
    for cfg in (CFG_A, CFG_B):
        H, R = cfg.H, cfg.R
        C2 = cfg.hids[2]
        out = np.zeros((2, T, C2, H, H), np.float32)
        for core, (b, cy, cx) in enumerate(CORE_ASSIGN):
            r = results[core][f'out{cfg.tag}']          # [T, n_hb, 128, R*R]
            r = r.reshape(T, C2, R, R)
            if cy:
                r = r[:, :, ::-1, :]
            if cx:
                r = r[:, :, :, ::-1]
            rs = slice(0, R) if cy == 0 else slice(H - R, H)
            cs = slice(0, R) if cx == 0 else slice(H - R, H)
            out[b, :, :, rs, cs] = r
        outs[cfg.tag] = out[:, :T0]
    return outs['A'], outs['B']


def kernel(**inputs):
    nc = _get_program()
    in_maps = _make_in_maps(inputs)
    res = run_bass_kernel_spmd(nc, in_maps, core_ids=list(range(8)))
    return _assemble(res.results, inputs)


if __name__ == "__main__":
    ins = {k: np.random.randn(*[2, 8, 512, 38, 38]).astype(np.float32)
           for k in []}
    print("kernel module ok")


# revision 19
# speedup vs baseline: 1.1134x; 1.0464x over previous
"""Trainium2 Bass kernel for nn_DecoderRNN_31550829756940.

Two independent 3-layer ConvLSTM stacks:
  stack A: x_23    (2,8,512,38,38), hidden (64,64,512)
  stack B: x_final (2,8,1024,19,19), hidden (64,64,1024)
Output: (relu(hA2_seq), relu(hB2_seq)).

Strategy (8 NeuronCores, one SPMD launch, no collectives):
  Each core owns one (batch, image-corner) shard of BOTH stacks: 2 batch x
  4 corners (2x2). A corner is normalized to the top-left by flipping the
  input image and conv-tap order host-side, so every core runs the identical
  program. Cross-core halo communication is replaced by shrinking redundant
  margins: at step t, layer l computes a valid square of side
  v_l(t) = min(H, R + (T-t) + (2-l)), which provides exactly the one-row/col
  halo the next step/layer needs. The conv is computed as 9 shifted matmuls
  accumulated in PSUM (channels-on-partitions, bf16 inputs, fp32 accum), with
  the 64-channel contractions packed pairwise into 128-partition k-tiles via
  shifted doubled buffers. Gate weights stream from HBM each step (they
  exceed SBUF); the LSTM cell runs on ACT (sigmoid/tanh w/ fused bias) + DVE.
"""
import os
import numpy as np
import ml_dtypes

import concourse.bacc as bacc
import concourse.bass as bass
import concourse.mybir as mybir
import concourse.tile as tile
from concourse.bass_utils import run_bass_kernel_spmd

F32 = mybir.dt.float32
BF16 = mybir.dt.float16 if os.environ.get("KDT", "fp16") == "fp16" else mybir.dt.bfloat16
NPDT = np.float16 if os.environ.get("KDT", "fp16") == "fp16" else ml_dtypes.bfloat16
AF = mybir.ActivationFunctionType

T = int(os.environ.get("KERNEL_T", "8"))
MAX_PSUM_N = 512


class Cfg:
    def __init__(self, tag, cin, H, R, hid2):
        self.tag = tag          # 'A' | 'B'
        self.cin = cin          # input channels (512 | 1024)
        self.H = H              # image side
        self.R = R              # owned corner side
        self.hids = [64, 64, hid2]
        self.n_cb = cin // 128
        self.sx = min(H, self.v(0, 1) + 1)   # shipped x side
        self.sxb = self.sx + 2
        self.hbuf = [self.v(l, 1) + 2 for l in range(3)]   # padded h buffer sides
        self.csz = [self.v(l, 1) for l in range(3)]        # c buffer sides
        self.M2 = 4 * hid2
        self.n_hb2 = hid2 // 128

    def v(self, l, t):
        return min(self.H, self.R + (T - t) + (2 - l))


CFG_A = Cfg('A', 512, 38, 19, 512)
CFG_B = Cfg('B', 1024, 19, 10, 1024)

# core -> (batch, flip_y, flip_x)
CORE_ASSIGN = [(b, cy, cx) for b in (0, 1) for cy in (0, 1) for cx in (0, 1)]


# ---------------------------------------------------------------- tap descs
def tap_descs(cin_x, hid):
    """k-tile descriptors for one layer's conv, x-taps first then h-taps.
    Each: (src, dy, dx, packs); packs = [(part0, nch, in_ch0, wdy, wdx), ...]
    src: ('x',cb) input x c-tile | ('pd',1|2) prev-layer doubled buf |
         ('od',1|2) own-h doubled buf | ('oh',cb) own-h full c-tile
    """
    descs = []

    def add_packed(srcmk, ch0):
        for dy in range(3):
            descs.append((srcmk(1), dy, 0,
                          [(0, 64, ch0, dy, 0), (64, 64, ch0, dy, 1)]))
        descs.append((srcmk(2), 0, 2, [(0, 64, ch0, 0, 2), (64, 64, ch0, 1, 2)]))
        descs.append((srcmk(1), 2, 2, [(0, 64, ch0, 2, 2)]))

    if cin_x == 64:
        add_packed(lambda j: ('pd', j), 0)
    else:
        for dy in range(3):
            for dx in range(3):
                for cb in range(cin_x // 128):
                    descs.append((('x', cb), dy, dx,
                                  [(0, 128, cb * 128, dy, dx)]))
    n_x = len(descs)
    if hid == 64:
        add_packed(lambda j: ('od', j), cin_x)
    else:
        for dy in range(3):
            for dx in range(3):
                for cb in range(hid // 128):
                    descs.append((('oh', cb), dy, dx,
                                  [(0, 128, cin_x + cb * 128, dy, dx)]))
    return descs, n_x


def pack_layer_w(w, cin_x, hid, flip_y, flip_x):
    """w: (4h, cin_x+hid, 3, 3) f32 -> [n_k, 128, 4h] bf16 (+bias layout)."""
    if flip_y:
        w = w[:, :, ::-1, :]
    if flip_x:
        w = w[:, :, :, ::-1]
    descs, _ = tap_descs(cin_x, hid)
    M = 4 * hid
    out = np.zeros((len(descs), 128, M), np.float32)
    for i, (_, _, _, packs) in enumerate(descs):
        for (p0, nch, c0, wdy, wdx) in packs:
            out[i, p0:p0 + nch, :] = w[:, c0:c0 + nch, wdy, wdx].T
    return np.ascontiguousarray(out.astype(NPDT))


def stream_repack_l2(w, hid):
    """[nk,128,4h] -> [n_hb, 128, nk*512] so a per-(hb, k-group) weight fetch
    is one DMA of long contiguous per-partition runs."""
    nk = w.shape[0]
    n_hb = hid // 128
    out = np.empty((n_hb, 128, nk, 4, 128), w.dtype)
    w5 = w.reshape(nk, 128, 4, n_hb, 128)          # [k,p,g,hb,c]
    out[:] = w5.transpose(3, 1, 0, 2, 4)           # [hb,p,k,g,c]
    return np.ascontiguousarray(out.reshape(n_hb, 128, nk * 512))


def stream_repack_l0(w):
    """[nk,128,256] -> [128, nk*256] (partition-major)."""
    return np.ascontiguousarray(w.transpose(1, 0, 2).reshape(128, -1))


def pack_bias(b, hid):
    """b: (4h,) -> [128, M/128] f32, col j = rows j*128..(j+1)*128."""
    M = 4 * hid
    nm = max(1, M // 128)
    return np.ascontiguousarray(b.reshape(nm, 128).T.astype(np.float32))


def chunks_of(v):
    """Split v rows into row-chunks with <= MAX_PSUM_N px each."""
    n = -(-(v * v) // MAX_PSUM_N)
    base, rem = divmod(v, n)
    out, r0 = [], 0
    for i in range(n):
        nr = base + (1 if i < rem else 0)
        out.append((r0, nr))
        r0 += nr
    return out


# ---------------------------------------------------------------- builder
def build_program():
    nc = bacc.Bacc("TRN2", target_bir_lowering=False, debug=False,
                   enable_asserts=False, num_devices=8)
    dram = {}
    for cfg in (CFG_A, CFG_B):
        t_ = cfg.tag
        dram[f'x{t_}'] = nc.dram_tensor(
            f'x{t_}', [T, cfg.n_cb, 128, cfg.sxb * cfg.sxb], BF16,
            kind="ExternalInput").ap()
        for l in range(3):
            cin_x = cfg.cin if l == 0 else 64
            nk = len(tap_descs(cin_x, cfg.hids[l])[0])
            M = 4 * cfg.hids[l]
            if l == 0:
                shape = [128, nk * 256]
            elif l == 1:
                shape = [nk, 128, 256]
            else:
                shape = [cfg.n_hb2, 128, nk * 512]
            dram[f'w{t_}{l}'] = nc.dram_tensor(
                f'w{t_}{l}', shape, BF16, kind="ExternalInput").ap()
            dram[f'b{t_}{l}'] = nc.dram_tensor(
                f'b{t_}{l}', [128, max(1, M // 128)], F32,
                kind="ExternalInput").ap()
        dram[f'out{t_}'] = nc.dram_tensor(
            f'out{t_}', [T, cfg.n_hb2, 128, cfg.R * cfg.R], F32,
            kind="ExternalOutput").ap()

    with tile.TileContext(nc) as tc:
        import contextlib
        with contextlib.ExitStack() as ctx:
            pers = ctx.enter_context(tc.tile_pool(name="pers", bufs=1))
            xpool = ctx.enter_context(tc.tile_pool(name="xpool", bufs=1))
            wstr = ctx.enter_context(tc.tile_pool(name="wstr", bufs=1))
            psp = ctx.enter_context(tc.tile_pool(name="ps", bufs=8, space="PSUM"))
            cellp = ctx.enter_context(tc.tile_pool(name="cellp", bufs=1))
            stgp = ctx.enter_context(tc.tile_pool(name="stgp", bufs=1))

            st = {}  # per-stack state tiles
            for cfg in (CFG_A, CFG_B):
                t_ = cfg.tag
                s = {}
                # doubled bufs for h0, h1 (64-channel layers)
                for l in (0, 1):
                    side = cfg.hbuf[l]
                    for j in (1, 2):
                        tl = pers.tile([128, side * side], BF16,
                                       name=f'h{t_}{l}d{j}', tag=f'h{t_}{l}d{j}')
                        nc.vector.memset(tl[:], 0.0)
                        s[f'h{l}d{j}'] = tl
                # full h2 buffers, ping-pong by step parity: a cell write of
                # block hb at step t must not clobber h(t-1) while later
                # hidden blocks' convs of the same step still read it.
                side = cfg.hbuf[2]
                s['h2'] = [[], []]
                for par in (0, 1):
                    for cb in range(cfg.n_hb2):
                        tl = pers.tile([128, side * side], BF16,
                                       name=f'h{t_}2p{par}_{cb}',
                                       tag=f'h{t_}2p{par}_{cb}')
                        nc.vector.memset(tl[:], 0.0)
                        s['h2'][par].append(tl)
                # c tiles
                s['c'] = []
                for l in range(3):
                    n_hb = max(1, cfg.hids[l] // 128)
                    cs = cfg.csz[l]
                    row = []
                    for hb in range(n_hb):
                        pw = 64 if cfg.hids[l] == 64 else 128
                        tl = pers.tile([pw, cs * cs], F32,
                                       name=f'c{t_}{l}_{hb}', tag=f'c{t_}{l}_{hb}')
                        row.append(tl)
                    s['c'].append(row)
                # biases
                s['bias'] = []
                for l in range(3):
                    nm = max(1, 4 * cfg.hids[l] // 128)
                    tl = pers.tile([128, nm], F32, name=f'bias{t_}{l}',
                                   tag=f'bias{t_}{l}')
                    nc.sync.dma_start(tl[:], dram[f'b{t_}{l}'][:])
                    s['bias'].append(tl)
                # resident l1 weights
                s['w1'] = []
                for k in range(len(tap_descs(64, 64)[0])):
                    tl = pers.tile([128, 256], BF16, name=f'w1{t_}_{k}',
                                   tag=f'w1{t_}_{k}')
                    nc.sync.dma_start(tl[:], dram[f'w{t_}1'][k])
                    s['w1'].append(tl)
                st[t_] = s

            # ---------------- one conv+cell layer step ----------------
            def layer_step(cfg, l, t, hbs=None):
                t_ = cfg.tag
                s = st[t_]
                hid = cfg.hids[l]
                cin_x = cfg.cin if l == 0 else 64
                descs, n_x = tap_descs(cin_x, hid)
                v = cfg.v(l, t)
                chs = chunks_of(v)
                n_hb = max(1, hid // 128)
                first_t = (t == 1)
                # h-taps first: they depend only on last step's h, so the PE
                # can start a layer's conv before the previous layer finishes;
                # only the trailing x-taps wait on the fresh input.
                kidx = (list(range(n_x)) if first_t else
                        list(range(n_x, len(descs))) + list(range(n_x)))
                wd = dram[f'w{t_}{l}']

                # source buffer resolution -> (tile, bufW)
                def src_buf(src):
                    if src[0] == 'x':
                        return s['xt'][src[1]], cfg.sxb
                    if src[0] == 'pd':
                        return s[f'h{l-1}d{src[1]}'], cfg.hbuf[l - 1]
                    if src[0] == 'od':
                        return s[f'h{l}d{src[1]}'], cfg.hbuf[l]
                    if src[0] == 'oh':
                        return s['h2'][(t - 1) % 2][src[1]], cfg.hbuf[2]
                    raise KeyError(src)

                if l == 0:
                    # stream x(t) c-tiles
                    s['xt'] = []
                    for cb in range(cfg.n_cb):
                        tl = xpool.tile([128, cfg.sxb * cfg.sxb], BF16,
                                        name=f'x{t_}_{t}_{cb}', tag=f'x{t_}',
                                        bufs=2 * cfg.n_cb + 2)
                        nc.sync.dma_start(tl[:], dram[f'x{t_}'][t - 1, cb])
                        s['xt'].append(tl)

                if hid == 64:
                    mlist = [0, 1]          # m-tiles [i;f], [o;g]
                else:
                    glist = [0, 2, 3] if first_t else [0, 1, 2, 3]

                for hb in (range(n_hb) if hbs is None else hbs):
                    # -------- weight fetch plan: grouped contiguous DMAs
                    wtiles = {}
                    if l == 1:
                        for k in kidx:
                            wtiles[k] = (s['w1'][k], 0)
                    else:
                        G = 8 if l == 0 else 4
                        wcols = 256 if l == 0 else 512
                        wtag = 'wstr0' if l == 0 else 'wstr2'
                        src_flat = wd if l == 0 else wd[hb]
                        # group only over ascending-contiguous runs of kidx
                        runs, cur = [], [kidx[0]]
                        for k in kidx[1:]:
                            if k == cur[-1] + 1:
                                cur.append(k)
                            else:
                                runs.append(cur)
                                cur = [k]
                        runs.append(cur)
                        groups = []
                        for run in runs:
                            for k0 in range(0, len(run), G):
                                groups.append(run[k0:k0 + G])
                        for ks in groups:
                            ng = len(ks)
                            wt = wstr.tile(
                                [128, ng * wcols], BF16,
                                name=f'w{t_}{l}t{t}h{hb}k{ks[0]}',
                                tag=wtag, bufs=8 if l == 2 else 5)
                            nc.sync.dma_start(
                                wt[:],
                                src_flat[:, ks[0] * wcols:
                                         (ks[0] + ng) * wcols])
                            for idx, k in enumerate(ks):
                                wtiles[k] = (wt, idx * wcols)
                    # -------- conv: accumulate psums over k-tiles
                    ps = {}
                    if hid == 64:
                        for ci in range(len(chs)):
                            for j in mlist:
                                ps[(j, ci)] = psp.tile(
                                    [128, chs[ci][1] * v], F32,
                                    name=f'ps{t_}{l}t{t}c{ci}m{j}', tag='ps')
                    else:
                        for ci in range(len(chs)):
                            for g in glist:
                                ps[(g, ci)] = psp.tile(
                                    [128, chs[ci][1] * v], F32,
                                    name=f'ps{t_}{l}t{t}h{hb}c{ci}g{g}', tag='ps')
                    for ii, k in enumerate(kidx):
                        src, dy, dx, _ = descs[k]
                        sbuf, bufW = src_buf(src)
                        rhs3 = sbuf[:].rearrange("p (r c) -> p r c", c=bufW)
                        wt, woff = wtiles[k]
                        start = (ii == 0)
                        stop = (ii == len(kidx) - 1)
                        for ci, (r0, nr) in enumerate(chs):
                            rhs = rhs3[:, dy + r0: dy + r0 + nr, dx: dx + v]
                            if hid == 64:
                                for j in mlist:
                                    nc.tensor.matmul(
                                        ps[(j, ci)][:],
                                        wt[:, woff + j * 128:woff + (j + 1) * 128],
                                        rhs, start=start, stop=stop)
                            else:
                                for g in glist:
                                    nc.tensor.matmul(
                                        ps[(g, ci)][:],
                                        wt[:, woff + g * 128:woff + (g + 1) * 128],
                                        rhs, start=start, stop=stop)

                    # -------- cell math per chunk
                    for ci, (r0, nr) in enumerate(chs):
                        n = nr * v
                        pw = 64 if hid == 64 else 128
                        bias = s['bias'][l]
                        if hid == 64:
                            i_ap = ps[(0, ci)][0:64, :]
                            f_ap = ps[(0, ci)][64:128, :]
                            o_ap = ps[(1, ci)][0:64, :]
                            g_ap = ps[(1, ci)][64:128, :]
                            b_i, b_f = bias[0:64, 0:1], bias[64:128, 0:1]
                            b_o, b_g = bias[0:64, 1:2], bias[64:128, 1:2]
                        else:
                            i_ap = ps[(0, ci)][:]
                            f_ap = None if first_t else ps[(1, ci)][:]
                            o_ap = ps[(2, ci)][:]
                            g_ap = ps[(3, ci)][:]
                            b_i = bias[:, 0 * n_hb + hb: 0 * n_hb + hb + 1]
                            b_f = bias[:, 1 * n_hb + hb: 1 * n_hb + hb + 1]
                            b_o = bias[:, 2 * n_hb + hb: 2 * n_hb + hb + 1]
                            b_g = bias[:, 3 * n_hb + hb: 3 * n_hb + hb + 1]
                        # c buffer has fixed row stride cs (layer's max side)
                        cs = cfg.csz[l]
                        c_ap = s['c'][l][hb][:].rearrange(
                            "p (r c) -> p r c", c=cs)[:, r0:r0 + nr, 0:v]
                        v3 = lambda ap: ap.rearrange("p (r c) -> p r c", c=v)
                        nm = f'{t_}{l}t{t}h{hb}c{ci}'

                        sig_i = cellp.tile([pw, n], F32, name=f'si{nm}',
                                           tag='cell', bufs=10)
                        nc.scalar.activation(sig_i[:], i_ap, AF.Sigmoid, bias=b_i)
                        tanh_g = cellp.tile([pw, n], F32, name=f'tg{nm}',
                                            tag='cell', bufs=10)
                        nc.scalar.activation(tanh_g[:], g_ap, AF.Tanh, bias=b_g)
                        if first_t:
                            nc.vector.tensor_mul(c_ap, v3(sig_i[:]), v3(tanh_g[:]))
                        else:
                            t1 = cellp.tile([pw, n], F32, name=f't1{nm}',
                                            tag='cell', bufs=10)
                            nc.vector.tensor_mul(t1[:], sig_i[:], tanh_g[:])
                            sig_f = cellp.tile([pw, n], F32, name=f'sf{nm}',
                                               tag='cell', bufs=10)
                            nc.scalar.activation(sig_f[:], f_ap, AF.Sigmoid,
                                                 bias=b_f)
                            t2 = cellp.tile([pw, n], F32, name=f't2{nm}',
                                            tag='cell', bufs=10)
                            nc.vector.tensor_mul(v3(t2[:]), v3(sig_f[:]), c_ap)
                            nc.vector.tensor_add(c_ap, v3(t1[:]), v3(t2[:]))
                        tanh_c = cellp.tile([pw, n], F32, name=f'tc{nm}',
                                            tag='cell', bufs=10)
                        nc.scalar.activation(v3(tanh_c[:]), c_ap, AF.Tanh)
                        sig_o = cellp.tile([pw, n], F32, name=f'so{nm}',
                                           tag='cell', bufs=10)
                        nc.scalar.activation(sig_o[:], o_ap, AF.Sigmoid, bias=b_o)
                        h_tmp = cellp.tile([pw, n], F32, name=f'ht{nm}',
                                           tag='cell', bufs=10)
                        nc.vector.tensor_mul(h_tmp[:], sig_o[:], tanh_c[:])
                        h3 = h_tmp[:].rearrange("p (r c) -> p r c", c=v)

                        # -------- h writes
                        if hid == 64:
                            side = cfg.hbuf[l]
                            d1 = s[f'h{l}d1'][:].rearrange(
                                "p (r c) -> p r c", c=side)
                            d2 = s[f'h{l}d2'][:].rearrange(
                                "p (r c) -> p r c", c=side)
                            nc.vector.tensor_copy(
                                d1[0:64, 1 + r0:1 + r0 + nr, 1:1 + v], h3)
                            nc.vector.tensor_copy(
                                d1[64:128, 1 + r0:1 + r0 + nr, 0:v], h3)
                            nc.vector.tensor_copy(
                                d2[0:64, 1 + r0:1 + r0 + nr, 1:1 + v], h3)
                            nc.vector.tensor_copy(
                                d2[64:128, r0:r0 + nr, 1:1 + v], h3)
                        else:
                            side = cfg.hbuf[2]
                            hdst = s['h2'][t % 2][hb][:].rearrange(
                                "p (r c) -> p r c", c=side)
                            nc.vector.tensor_copy(
                                hdst[:, 1 + r0:1 + r0 + nr, 1:1 + v], h3)
                            # -------- output: relu of owned region
                            if l == 2:
                                R = cfg.R
                                nr_own = max(0, min(r0 + nr, R) - r0)
                                if nr_own > 0:
                                    stg = stgp.tile([128, nr_own * R], F32,
                                                    name=f'st{nm}',
                                                    tag=f'stg{t_}', bufs=4)
                                    nc.vector.tensor_scalar_max(
                                        stg[:].rearrange("p (r c) -> p r c", c=R),
                                        h3[:, 0:nr_own, 0:R], 0.0)
                                    nc.sync.dma_start(
                                        dram[f'out{t_}'][t - 1, hb]
                                        [:, r0 * R:(r0 + nr_own) * R], stg[:])

            # Emission order = engine program order (in-order execution). B's
            # l2 weight stream can't be hidden by its own shrinking compute at
            # late steps, so interleave its hidden-block units with stack A's
            # (independent) work within each step to cover the DMA deficits.
            for t in range(1, T + 1):
                layer_step(CFG_B, 0, t)
                layer_step(CFG_B, 1, t)
                a_units = ([lambda tt=t: layer_step(CFG_A, 0, tt),
                            lambda tt=t: layer_step(CFG_A, 1, tt)] +
                           [lambda tt=t, h=hb: layer_step(CFG_A, 2, tt, [h])
                            for hb in range(CFG_A.n_hb2)])
                b_units = [lambda tt=t, h=hb: layer_step(CFG_B, 2, tt, [h])
                           for hb in range(CFG_B.n_hb2)]
                order = []
                na, nb = len(a_units), len(b_units)
                ia = ib = 0
                while ia < na or ib < nb:
                    if ib < nb:
                        order.append(b_units[ib]); ib += 1
                    if ia < na:
                        order.append(a_units[ia]); ia += 1
                for u in order:
                    u()

    nc.compile()
    return nc


# ---------------------------------------------------------------- host side
def _prep_x(x_b, cfg, flip_y, flip_x):
    """x_b: (T0, C, H, H) f32 for one batch el. -> [T, n_cb, 128, sxb*sxb] bf16"""
    x = x_b
    if flip_y:
        x = x[:, :, ::-1, :]
    if flip_x:
        x = x[:, :, :, ::-1]
    x = x[:, :, :cfg.sx, :cfg.sx]
    out = np.zeros((T, cfg.n_cb, 128, cfg.sxb, cfg.sxb), np.float32)
    xr = x.reshape(x.shape[0], cfg.n_cb, 128, cfg.sx, cfg.sx)
    out[:, :, :, 1:1 + cfg.sx, 1:1 + cfg.sx] = xr[:T]
    return np.ascontiguousarray(
        out.reshape(T, cfg.n_cb, 128, -1).astype(NPDT))


_PROG_CACHE = {}


def _get_program():
    if 'nc' not in _PROG_CACHE:
        _PROG_CACHE['nc'] = build_program()
    return _PROG_CACHE['nc']


def _make_in_maps(inputs):
    wsets = {}   # (stack, flip_y, flip_x) -> dict of weight arrays
    for (cy, cx) in [(0, 0), (0, 1), (1, 0), (1, 1)]:
        for cfg, pre in ((CFG_A, 'a'), (CFG_B, 'b')):
            d = {}
            for l in range(3):
                w = np.asarray(inputs[f'w_{pre}{l}'], np.float32)
                cin_x = cfg.cin if l == 0 else 64
                wk = pack_layer_w(w, cin_x, cfg.hids[l], bool(cy), bool(cx))
                if l == 0:
                    wk = stream_repack_l0(wk)
                elif l == 2:
                    wk = stream_repack_l2(wk, cfg.hids[l])
                d[f'w{cfg.tag}{l}'] = wk
                d[f'b{cfg.tag}{l}'] = pack_bias(
                    np.asarray(inputs[f'b_{pre}{l}'], np.float32), cfg.hids[l])
            wsets[(cfg.tag, cy, cx)] = d

    in_maps = []
    xA = np.asarray(inputs['x_23'], np.float32)
    xB = np.asarray(inputs['x_final'], np.float32)
    for (b, cy, cx) in CORE_ASSIGN:
        m = {}
        m['xA'] = _prep_x(xA[b], CFG_A, bool(cy), bool(cx))
        m['xB'] = _prep_x(xB[b], CFG_B, bool(cy), bool(cx))
        m.update(wsets[('A', cy, cx)])
        m.update(wsets[('B', cy, cx)])
        in_maps.append(m)
    return in_maps


def _assemble(results, inputs):
    xA = np.asarray(inputs['x_23'])
    T0 = xA.shape[1]
    outs = {}
    for cfg in (CFG_A, CFG_B):
        H, R = cfg.H, cfg.R
        C2 = cfg.hids[2]
        out = np.zeros((2, T, C2, H, H), np.float32)
        for core, (b, cy, cx) in enumerate(CORE_ASSIGN):
            r = results[core][f'out{cfg.tag}']          # [T, n_hb, 128, R*R]
            r = r.reshape(T, C2, R, R)
            if cy:
                r = r[:, :, ::-1, :]
            if cx:
                r = r[:, :, :, ::-1]
            rs = slice(0, R) if cy == 0 else slice(H - R, H)
            cs = slice(0, R) if cx == 0 else slice(H - R, H)
            out[b, :, :, rs, cs] = r
        outs[cfg.tag] = out[:, :T0]
    return outs['A'], outs['B']


def kernel(**inputs):
    nc = _get_program()
    in_maps = _make_in_maps(inputs)
    res = run_bass_kernel_spmd(nc, in_maps, core_ids=list(range(8)))
    return _assemble(res.results, inputs)


if __name__ == "__main__":
    ins = {k: np.random.randn(*[2, 8, 512, 38, 38]).astype(np.float32)
           for k in []}
    print("kernel module ok")
